# revision 1
# baseline (speedup 1.0000x reference)
"""Distributed brute-force KNN (IndexFlatL2, K=3) + mean of gathered pred values.

Strategy (data-parallel over the memory bank N, queries replicated):
  - Host sorts the memory rows by ||m||^2 and shards the sorted bank across
    the 8 cores (12500 rows each), transposed so the PE moving operand
    [K=d, N=n] streams straight from DRAM.
  - Phase 1 (device): c[b, n] = (2q).m_n via fp8e4m3 DoubleRow matmuls
    (0.5 PE cycles/column, contraction pairs of k-subtiles) into fp32 PSUM.
    DVE tensor_reduce window-maxes each PSUM block directly (windows of 10
    columns); because rows are msq-sorted, ||m||^2 is constant to ~0.05
    within a window, so the window's best score s' = 2q.m - ||m||^2 is
    recovered as wmax(c) - msq_window (one [128, 1250] subtract per query
    chunk). max8 + max_index over each 50-window segment of the corrected
    window scores return each query's top-8 windows per segment (NSEG=25
    segments -> 200 candidate windows per core). The segment-level rank
    budget keeps the capture margin (~40+) far above the worst-case fp8
    score error (~10), so the true top-3 rows always survive inside the
    returned windows.
  - Phase 2 (host): rank the 1600 candidate windows per query, take the
    top WSEL, exactly re-score their rows (fp64), take the true top-3,
    gather pred_values (through the sort permutation), return the mean.
"""

import sys
import types

import ml_dtypes
import numpy as np

try:  # bass_utils' axon trace path imports this unconditionally when
    import antenv.axon_hooks  # noqa: F401  # BASS_TRACE is set; stub it if absent
except ImportError:
    _stub = types.ModuleType("antenv.axon_hooks")
    _stub.get_axon_ntff_profile_hook = lambda: None
    _stub.set_axon_ntff_profile_hook = lambda hook: None
    sys.modules["antenv.axon_hooks"] = _stub

import concourse.bacc as bacc
import concourse.mybir as mybir
import concourse.tile as tile
from concourse import bass_utils

B = 1024            # queries
D = 1024            # embedding dim
N = 100000          # memory rows
NCORES = 8
NS = N // NCORES    # 12500 memory rows per core
BLK = 500           # matmul free-dim tile (fits one PSUM bank in fp32)
NBLK = NS // BLK    # 25 blocks per core
GROUP_W = 5         # blocks fetched per DMA group (25 = 5 uniform groups)
KT = D // 128       # 8 contraction tiles
BCH = B // 128      # 8 query chunks of 128
WND = 10            # window width for the DVE windowed max
NWIN = NS // WND    # 1250 windows per core
WPB = BLK // WND    # 50 windows per block
TOPB = 8            # DVE max8 width
NSEG = 25           # window segments per core; top-8 windows per segment
SEGW = NWIN // NSEG  # 250 windows per segment
NCAND = NSEG * TOPB  # 40 candidate windows per query per core
K = 3
WSEL = 32           # windows exactly re-scored on host per query

USE_FP8 = True      # False falls back to fp16 matmuls (k-step 1)

_CACHE = {}
LAST_RUN = None
LAST_TOP_IDX = None


def _build_program(nblk=NBLK, bch=BCH, group_w=GROUP_W):
    nc = bacc.Bacc(
        "TRN2",
        target_bir_lowering=False,
        debug=False,
        enable_asserts=False,
        num_devices=NCORES,
    )
    f32 = mybir.dt.float32
    u32 = mybir.dt.uint32
    mmdt = mybir.dt.float8e4 if USE_FP8 else mybir.dt.float16
    kstep = 2 if USE_FP8 else 1
    perf_mode = mybir.MatmulPerfMode.DoubleRow if USE_FP8 else None
    ns = nblk * BLK
    nwin = ns // WND
    b = bch * 128

    mT = nc.dram_tensor("mT", [D, ns], mmdt, kind="ExternalInput").ap()
    qT = nc.dram_tensor("qT", [D, b], mmdt, kind="ExternalInput").ap()
    msqw = nc.dram_tensor("msqw", [1, nwin], f32, kind="ExternalInput").ap()
    out_vals = nc.dram_tensor("out_vals", [b, NCAND], f32, kind="ExternalOutput").ap()
    out_idx = nc.dram_tensor("out_idx", [b, NCAND], u32, kind="ExternalOutput").ap()

    mT_r = mT.rearrange("(o p) n -> p o n", p=128)
    qT_r = qT.rearrange("(o p) b -> p o b", p=128)
    ov_r = out_vals.rearrange("(c p) j -> p c j", p=128)
    oi_r = out_idx.rearrange("(c p) j -> p c j", p=128)

    groups = []
    g0 = 0
    while g0 < nblk:
        w = min(group_w, nblk - g0)
        groups.append((g0, w))
        g0 += w

    with tile.TileContext(nc) as tc:
        with (
            tc.tile_pool(name="const", bufs=1) as cpool,
            tc.tile_pool(name="mov", bufs=2) as movpool,
            tc.tile_pool(name="wsc", bufs=3) as wscpool,
            tc.tile_pool(name="psum", bufs=8, space="PSUM") as pspool,
        ):
            qt_sb = cpool.tile([128, KT, b], mmdt, tag="qt")
            nc.sync.dma_start(qt_sb, qT_r)
            msqw_bc = cpool.tile([128, nwin], f32, tag="msqwbc")
            nc.sync.dma_start(msqw_bc, msqw.to_broadcast([128, nwin]))
            wmax = cpool.tile([128, bch, nwin], f32, tag="wmax")
            cand_v = cpool.tile([128, bch, NCAND], f32, tag="cv")
            cand_i = cpool.tile([128, bch, NCAND], u32, tag="ci")

            for blk0, w in groups:
                n0 = blk0 * BLK
                wn = w * BLK
                mov = movpool.tile([128, KT, group_w * BLK], mmdt, tag="mov")
                nc.sync.dma_start(mov[:, :, :wn], mT_r[:, :, n0 : n0 + wn])
                for bc in range(bch):
                    psums = [
                        pspool.tile([128, BLK], f32, tag="mm", name="mm_ps")
                        for _ in range(w)
                    ]
                    for k in range(0, KT, kstep):
                        lhsT = qt_sb[:, k : k + kstep, bc * 128 : (bc + 1) * 128]
                        for j in range(w):
                            nc.tensor.matmul(
                                psums[j],
                                lhsT=lhsT,
                                rhs=mov[:, k : k + kstep, j * BLK : (j + 1) * BLK],
                                start=(k == 0),
                                stop=(k + kstep >= KT),
                                perf_mode=perf_mode,
                            )
                    for j in range(w):
                        blk = blk0 + j
                        nc.vector.tensor_reduce(
                            wmax[:, bc, blk * WPB : (blk + 1) * WPB],
                            psums[j].rearrange("p (w t) -> p w t", t=WND),
                            axis=mybir.AxisListType.X,
                            op=mybir.AluOpType.max,
                            opt_input=False,
                        )
            segw = nwin // NSEG
            for bc in range(bch):
                wsc = wscpool.tile([128, nwin], f32, tag="wsc", name="wsc")
                nc.vector.tensor_sub(wsc, wmax[:, bc, :], msqw_bc)
                for f in range(NSEG):
                    seg = wsc[:, f * segw : (f + 1) * segw]
                    cv = cand_v[:, bc, f * TOPB : (f + 1) * TOPB]
                    nc.vector.max(out=cv, in_=seg)
                    nc.vector.max_index(
                        out=cand_i[:, bc, f * TOPB : (f + 1) * TOPB],
                        in_max=cv,
                        in_values=seg,
                    )
            nc.sync.dma_start(ov_r, cand_v)
            nc.sync.dma_start(oi_r, cand_i)
    nc.compile()
    return nc


def kernel(h_query, memory_embeds, pred_values):
    global LAST_RUN, LAST_TOP_IDX
    q = np.ascontiguousarray(np.asarray(h_query, dtype=np.float32))
    m = np.ascontiguousarray(np.asarray(memory_embeds, dtype=np.float32))
    pv = np.asarray(pred_values, dtype=np.float32)

    msq_full = np.einsum("nd,nd->n", m, m)
    perm = np.argsort(msq_full, kind="stable")
    m_s = m[perm]                      # msq-sorted memory bank
    msq_s = msq_full[perm]

    mmdt_np = ml_dtypes.float8_e4m3 if USE_FP8 else np.float16
    qTs = (np.ascontiguousarray(q.T) * np.float32(2.0)).astype(mmdt_np)
    mTs = np.ascontiguousarray(m_s.T).astype(mmdt_np)
    msqw_all = msq_s.reshape(N // WND, WND).mean(axis=1).astype(np.float32)

    if "nc" not in _CACHE:
        _CACHE["nc"] = _build_program()
    nc = _CACHE["nc"]

    in_maps = []
    for c in range(NCORES):
        sl = slice(c * NS, (c + 1) * NS)
        wsl = slice(c * NWIN, (c + 1) * NWIN)
        in_maps.append(
            {
                "mT": np.ascontiguousarray(mTs[:, sl]),
                "qT": qTs,
                "msqw": np.ascontiguousarray(msqw_all[wsl]).reshape(1, NWIN),
            }
        )

    res = bass_utils.run_bass_kernel_spmd(nc, in_maps, core_ids=list(range(NCORES)))
    LAST_RUN = res
    results = res.results

    # windows: value [B, 40] + in-segment index [B, 40] per core; global
    # window id = core*NWIN + seg*SEGW + idx; window w covers sorted rows
    # [w*WND, +WND).
    seg_off = (np.arange(NCAND, dtype=np.int64) // TOPB) * SEGW
    vals = np.concatenate([r["out_vals"] for r in results], axis=1)
    widx = np.concatenate(
        [
            r["out_idx"].astype(np.int64) + seg_off[None, :] + c * NWIN
            for c, r in enumerate(results)
        ],
        axis=1,
    )

    # Phase 2: pick top-WSEL windows per query, exactly re-score their rows.
    sel = np.argpartition(-vals, WSEL, axis=1)[:, :WSEL]
    wsel = np.take_along_axis(widx, sel, axis=1)           # [B, WSEL]
    rows = wsel[:, :, None] * WND + np.arange(WND)[None, None, :]
    cidx = rows.reshape(B, WSEL * WND)                     # sorted-space rows
    mg = m_s[cidx].astype(np.float64)                      # [B, WSEL*WND, D]
    s_exact = 2.0 * np.einsum("bd,bkd->bk", q.astype(np.float64), mg)
    s_exact -= np.einsum("bkd,bkd->bk", mg, mg)
    pick = np.argpartition(-s_exact, K, axis=1)[:, :K]
    top_sorted = np.take_along_axis(cidx, pick, axis=1)
    top_idx = perm[top_sorted]                             # original row ids
    LAST_TOP_IDX = top_idx
    y = pv[top_idx].astype(np.float64).mean()
    return np.float32(y)



# revision 2
# speedup vs baseline: 1.3994x; 1.3994x over previous
"""Distributed brute-force KNN (IndexFlatL2, K=3) + mean of gathered pred values.

Strategy (data-parallel over the memory bank N, queries replicated):
  - Device phase: per core, fp8e4m3 DoubleRow matmuls compute the corrected
    score s[b, n] = 2q.m_n - ||m_n||^2 directly in fp32 PSUM: the last 4 of
    the 1024 contraction dims are repurposed as a base-(256,32,4,0.5) digit
    encoding of -||m||^2 (digits are small integers, exact in fp8; residual
    <= 0.25; the 4 dropped data dims add ~N(0,4) noise, absorbed by the
    filter margin).  The DVE window-maxes each PSUM block (windows of 10
    rows) and the per-window maxima [B, 1250] stream back to DRAM.  No
    other device-side work: the Vector engine load (~136us) hides under the
    Tensor engine floor (~172us).
  - Host phase: rank the 8*1250 = 10000 candidate windows per query, take
    the top WSEL, exactly re-score their rows (fp64), take the true top-3,
    gather pred_values, return the mean.  Window capture margin (~50+ score
    units) dwarfs the fp8 scoring noise (~10).
"""

import sys
import types

import ml_dtypes
import numpy as np

try:  # bass_utils' axon trace path imports this unconditionally when
    import antenv.axon_hooks  # noqa: F401  # BASS_TRACE is set; stub it if absent
except ImportError:
    _stub = types.ModuleType("antenv.axon_hooks")
    _stub.get_axon_ntff_profile_hook = lambda: None
    _stub.set_axon_ntff_profile_hook = lambda hook: None
    sys.modules["antenv.axon_hooks"] = _stub

import concourse.bacc as bacc
import concourse.mybir as mybir
import concourse.tile as tile
from concourse import bass_utils

B = 1024            # queries
D = 1024            # embedding dim
N = 100000          # memory rows
NCORES = 8
NS = N // NCORES    # 12500 memory rows per core
BLK = 500           # matmul free-dim tile (fits one PSUM bank in fp32)
NBLK = NS // BLK    # 25 blocks per core
KT = D // 128       # 8 contraction subtiles
BCH = B // 128      # 8 query chunks of 128
WND = 10            # window width for the DVE windowed max
NWIN = NS // WND    # 1250 windows per core
WPB = BLK // WND    # 50 windows per block
K = 3
WSEL = 40           # windows exactly re-scored on host per query
# staged group widths: small first group so the PE starts ~1.5us in
GROUPS = [(0, 1), (1, 4), (5, 5), (10, 5), (15, 5), (20, 5)]
MAXW = 5

_CACHE = {}
LAST_RUN = None
LAST_TOP_IDX = None


def _build_program():
    nc = bacc.Bacc(
        "TRN2",
        target_bir_lowering=False,
        debug=False,
        enable_asserts=False,
        num_devices=NCORES,
    )
    f32 = mybir.dt.float32
    fp8 = mybir.dt.float8e4

    mT = nc.dram_tensor("mT", [D, NS], fp8, kind="ExternalInput").ap()
    qT = nc.dram_tensor("qT", [D, B], fp8, kind="ExternalInput").ap()
    out_w = nc.dram_tensor("out_w", [B, NWIN], f32, kind="ExternalOutput").ap()

    mT_r = mT.rearrange("(o p) n -> p o n", p=128)
    qT_r = qT.rearrange("(o p) b -> p o b", p=128)
    ow_r = out_w.rearrange("(c p) j -> p c j", p=128)

    with tile.TileContext(nc) as tc:
        with (
            tc.tile_pool(name="const", bufs=1) as cpool,
            tc.tile_pool(name="mov", bufs=2) as movpool,
            tc.tile_pool(name="wm", bufs=4) as wmpool,
            tc.tile_pool(name="psum", bufs=8, space="PSUM") as pspool,
        ):
            qt_sb = cpool.tile([128, KT, B], fp8, tag="qt")
            nc.sync.dma_start(qt_sb, qT_r)

            for g0, w in GROUPS:
                n0 = g0 * BLK
                wn = w * BLK
                mov = movpool.tile([128, KT, MAXW * BLK], fp8, tag="mov")
                nc.sync.dma_start(mov[:, :, :wn], mT_r[:, :, n0 : n0 + wn])
                for bc in range(BCH):
                    psums = [
                        pspool.tile([128, BLK], f32, tag="mm", name="mm_ps")
                        for _ in range(w)
                    ]
                    for k in range(0, KT, 2):
                        lhsT = qt_sb[:, k : k + 2, bc * 128 : (bc + 1) * 128]
                        for j in range(w):
                            nc.tensor.matmul(
                                psums[j],
                                lhsT=lhsT,
                                rhs=mov[:, k : k + 2, j * BLK : (j + 1) * BLK],
                                start=(k == 0),
                                stop=(k + 2 >= KT),
                                perf_mode=mybir.MatmulPerfMode.DoubleRow,
                            )
                    wt = wmpool.tile([128, MAXW * WPB], f32, tag="wt", name="wt")
                    for j in range(w):
                        nc.vector.tensor_reduce(
                            wt[:, j * WPB : (j + 1) * WPB],
                            psums[j].rearrange("p (w t) -> p w t", t=WND),
                            axis=mybir.AxisListType.X,
                            op=mybir.AluOpType.max,
                            opt_input=False,
                        )
                    nc.sync.dma_start(
                        ow_r[:, bc, g0 * WPB : (g0 + w) * WPB], wt[:, : w * WPB]
                    )
    nc.compile()
    return nc


def kernel(h_query, memory_embeds, pred_values):
    global LAST_RUN, LAST_TOP_IDX
    q = np.ascontiguousarray(np.asarray(h_query, dtype=np.float32))
    m = np.ascontiguousarray(np.asarray(memory_embeds, dtype=np.float32))
    pv = np.asarray(pred_values, dtype=np.float32)

    # -||m||^2 folded into the contraction as 4 digit rows (exact to 0.25)
    msq = np.einsum("nd,nd->n", m.astype(np.float64), m.astype(np.float64))
    a = np.rint(msq / 256.0)
    r = msq - 256.0 * a
    b = np.rint(r / 32.0)
    r -= 32.0 * b
    c = np.rint(r / 4.0)
    r -= 4.0 * c
    d = np.rint(r / 0.5)
    digit_rows = np.stack([-a, -b, -c, -d]).astype(np.float32)  # [4, N]

    fp8 = ml_dtypes.float8_e4m3
    qTs = np.empty((D, B), dtype=fp8)
    qTs[: D - 4] = (q.T[: D - 4] * np.float32(2.0)).astype(fp8)
    qTs[D - 4 :] = np.array([256.0, 32.0, 4.0, 0.5], dtype=np.float32)[
        :, None
    ].astype(fp8)
    mTs = np.empty((D, N), dtype=fp8)
    mTs[: D - 4] = m.T[: D - 4].astype(fp8)
    mTs[D - 4 :] = digit_rows.astype(fp8)

    if "nc" not in _CACHE:
        _CACHE["nc"] = _build_program()
    nc = _CACHE["nc"]

    in_maps = []
    for cix in range(NCORES):
        sl = slice(cix * NS, (cix + 1) * NS)
        in_maps.append({"mT": np.ascontiguousarray(mTs[:, sl]), "qT": qTs})

    res = bass_utils.run_bass_kernel_spmd(nc, in_maps, core_ids=list(range(NCORES)))
    LAST_RUN = res
    results = res.results

    # [B, 8*1250] corrected window scores; window w covers rows
    # [(w // NWIN) * NS + (w % NWIN) * WND, +WND)
    wall = np.concatenate([r["out_w"] for r in results], axis=1)

    sel = np.argpartition(-wall, WSEL, axis=1)[:, :WSEL]      # [B, WSEL]
    core = sel // NWIN
    rows = (core * NS + (sel % NWIN) * WND)[:, :, None] + np.arange(WND)[
        None, None, :
    ]
    cidx = rows.reshape(B, WSEL * WND)                         # candidate rows

    # exact fp64 re-score of candidate rows, chunked over queries
    q64 = q.astype(np.float64)
    m64 = m.astype(np.float64)
    msq64 = msq
    top_idx = np.empty((B, K), dtype=np.int64)
    CH = 128
    for b0 in range(0, B, CH):
        ci = cidx[b0 : b0 + CH]                                # [CH, WSEL*WND]
        mg = m64[ci]                                           # [CH, R, D]
        s = 2.0 * np.einsum("bd,bkd->bk", q64[b0 : b0 + CH], mg)
        s -= msq64[ci]
        pick = np.argpartition(-s, K, axis=1)[:, :K]
        top_idx[b0 : b0 + CH] = np.take_along_axis(ci, pick, axis=1)
    LAST_TOP_IDX = top_idx
    y = pv[top_idx].astype(np.float64).mean()
    return np.float32(y)
